# revision 1
# baseline (speedup 1.0000x reference)
"""Bilateral filter (7x7, dilation 1) Trainium2 Bass kernel.

Problem: input [2, 18, 1024, 1024] f32.
  filterable = input[:, :8]; params = -(input[:, 8:]**2)
  range coeffs = params[:, :8], sx = params[:, 8], sy = params[:, 9]
  out[c] = sum_taps w * f_c(shifted) / sum_taps w, c < 3
  w = exp(sum_c r_c (fn_c - f_c)^2 + sx dx^2 + sy dy^2), OOB taps masked.

Sharding: data-parallel over (batch, H): 8 cores, each gets 256 rows of one
batch image (+3 halo rows each side, sentinel-padded host-side).  Out-of-image
taps get weight exactly 0 because the sentinel (1e18) drives the quadratic
form to -huge and exp underflows to +0.

Per-core layout: H rows on partitions (128 x 2 blocks), W in chunks of 256 on
the free axis with the 8 filterable channels interleaved (x*8+c).  Row shifts
(oy) come from 7 row-shifted tile copies; column shifts (ox) are free-axis
offsets into the 6-column halo.

Engine split per tap: DVE sub/reduce/adds, ACT square/exp, GPSIMD r*d^2.
"""

import sys

if "/opt/trn_rl_repo" not in sys.path:
    sys.path.insert(0, "/opt/trn_rl_repo")

import numpy as np

import concourse.bass as bass
import concourse.mybir as mybir
from concourse.bacc import Bacc
from concourse.tile import TileContext

FP32 = mybir.dt.float32

B, C_ALL, H, W = 2, 18, 1024, 1024
CF = 8                      # filterable channels
CO = 3                      # output channels
KS, RAD = 7, 3
HC = H * B // 8             # 256 output rows per core
HIN = HC + 2 * RAD          # 262 input rows per core (halo padded host-side)
WC = 256                    # W chunk
NW = W // WC                # 4
NHB = HC // 128             # 2
SENT = 1.0e18               # sentinel padding value -> tap weight exp(-huge)=0
D2 = [9.0, 4.0, 1.0, 0.0, 1.0, 4.0, 9.0]   # (k-3)^2 for k in 0..6
D2IDX = [3, 2, 1, 0, 1, 2, 3]              # index into [0,1,4,9]
D2VALS = [0.0, 1.0, 4.0, 9.0]

_CACHED = {}
TAP_SET = None   # optional [(i,j)] subset for debugging


def _ilv(ap, n, c=CF):
    """View flat [128, n*c] region as [128, n, c] (channel-interleaved)."""
    return ap.rearrange("p (x c) -> p x c", c=c)


def build_nc(macros=None):
    nc = Bacc()
    x = nc.dram_tensor("x", [C_ALL, HIN, W], FP32, kind="ExternalInput")
    y = nc.dram_tensor("y", [CO, HC, W], FP32, kind="ExternalOutput")

    if macros is None:
        macros = [(hb, wck) for hb in range(NHB) for wck in range(NW)]
    with TileContext(nc) as tc:
        with (
            tc.tile_pool(name="fpool", bufs=1) as fpool,
            tc.tile_pool(name="cpool", bufs=1) as cpool,
            tc.tile_pool(name="dpool", bufs=5) as dpool,
            tc.tile_pool(name="spool", bufs=4) as spool,
            tc.tile_pool(name="ppool", bufs=1, space="PSUM") as ppool,
        ):
            for hb, wcki in macros:
                _macro(nc, tc, x, y, fpool, cpool, dpool, spool, ppool, hb, wcki)
    nc.compile()
    return nc


def _macro(nc, tc, x, y, fpool, cpool, dpool, spool, ppool, hb, wck):
    w0 = wck * WC
    r0 = hb * 128
    wtile = WC + 2 * RAD
    # tile col t  <->  image col w0 - 3 + t
    lo = RAD if wck == 0 else 0
    hi = wtile - RAD if wck == NW - 1 else wtile

    # ---- load + interleave the 7 row-shifted filterable tile sets ----
    F = []
    for oy in range(KS):
        Fi = fpool.tile([128, wtile * CF], FP32, tag=f"F{oy}", bufs=1,
                        name=f"F{oy}_{hb}_{wck}")
        for c in range(CF):
            pl = fpool.tile([128, wtile], FP32, tag="pl", bufs=3,
                            name=f"pl_{hb}_{wck}_{oy}_{c}")
            if lo > 0:
                nc.gpsimd.memset(pl[:, 0:lo], SENT)
            if hi < wtile:
                nc.gpsimd.memset(pl[:, hi:wtile], SENT)
            nc.sync.dma_start(
                out=pl[:, lo:hi],
                in_=x[c, r0 + oy : r0 + oy + 128, w0 - RAD + lo : w0 - RAD + hi],
            )
            # interleave: Fi[p, t*8+c] = pl[p, t]   (ACT, strided out)
            nc.scalar.copy(_ilv(Fi[:], wtile)[:, :, c], pl[:])
        F.append(Fi)
    Fc = _ilv(F[RAD][:, RAD * CF : (RAD + WC) * CF], WC)      # center view

    # ---- params: R (interleaved), sx2, sy2 ----
    R = cpool.tile([128, WC * CF], FP32, tag="R", name=f"R_{hb}_{wck}")
    for c in range(CF):
        pp = fpool.tile([128, WC], FP32, tag="pp", bufs=2,
                        name=f"pp_{hb}_{wck}_{c}")
        nc.sync.dma_start(
            out=pp[:], in_=x[CF + c, r0 + RAD : r0 + RAD + 128, w0 : w0 + WC])
        nc.vector.scalar_tensor_tensor(
            _ilv(R[:], WC)[:, :, c], pp[:], -1.0, pp[:],
            mybir.AluOpType.mult, mybir.AluOpType.mult)
    sxy2 = cpool.tile([128, 2 * WC], FP32, tag="sxy2", name=f"sxy2_{hb}_{wck}")
    for k in range(2):
        pp = fpool.tile([128, WC], FP32, tag="pp", bufs=2,
                        name=f"pps_{hb}_{wck}_{k}")
        nc.sync.dma_start(
            out=pp[:], in_=x[2 * CF + k, r0 + RAD : r0 + RAD + 128, w0 : w0 + WC])
        nc.vector.scalar_tensor_tensor(
            sxy2[:, k * WC : (k + 1) * WC], pp[:], -1.0, pp[:],
            mybir.AluOpType.mult, mybir.AluOpType.mult)
    sx2 = sxy2[:, 0:WC]
    sy2 = sxy2[:, WC : 2 * WC]

    # ---- Asp[a][b] = a*sx2 + b*sy2  (spatial log-weight, 16 combos) ----
    Ab = spool.tile([128, 4 * WC], FP32, tag="Ab", bufs=2, name=f"Ab_{hb}_{wck}")
    for bi, bval in enumerate(D2VALS):
        nc.vector.tensor_scalar_mul(
            Ab[:, bi * WC : (bi + 1) * WC], sy2, float(bval))
    Asp = cpool.tile([128, 16 * WC], FP32, tag="Asp", name=f"Asp_{hb}_{wck}")
    for ai, aval in enumerate(D2VALS):
        for bi in range(4):
            nc.vector.scalar_tensor_tensor(
                Asp[:, (ai * 4 + bi) * WC : (ai * 4 + bi + 1) * WC],
                sx2, float(aval), Ab[:, bi * WC : (bi + 1) * WC],
                mybir.AluOpType.mult, mybir.AluOpType.add)

    # ---- accumulators ----
    acc = cpool.tile([128, WC * CO], FP32, tag="acc", name=f"acc_{hb}_{wck}")
    wsum = cpool.tile([128, WC], FP32, tag="wsum", name=f"wsum_{hb}_{wck}")
    nc.gpsimd.memset(acc[:], 0.0)
    nc.gpsimd.memset(wsum[:], 0.0)

    # ---- 49 taps ----
    taps = TAP_SET if TAP_SET is not None else [(i, j) for i in range(KS) for j in range(KS)]
    for i, j in taps:            # oy = i - 3, ox = j - 3
        if True:
            Fi = F[i]
            sh = _ilv(Fi[:, j * CF : (j + WC) * CF], WC)     # shifted read
            d = dpool.tile([128, WC * CF], FP32, tag="d",
                           name=f"d_{hb}_{wck}_{i}_{j}")
            nc.vector.tensor_sub(_ilv(d[:], WC), sh, Fc)
            nc.scalar.activation(d[:], d[:], mybir.ActivationFunctionType.Square)
            nc.gpsimd.tensor_mul(d[:], R[:], d[:])
            s = spool.tile([128, WC], FP32, tag="s",
                           name=f"s_{hb}_{wck}_{i}_{j}")
            nc.vector.tensor_reduce(s[:], _ilv(d[:], WC),
                                    axis=mybir.AxisListType.X,
                                    op=mybir.AluOpType.add)
            k = (D2IDX[j] * 4 + D2IDX[i]) * WC
            nc.vector.tensor_add(s[:], s[:], Asp[:, k : k + WC])
            w_t = spool.tile([128, WC], FP32, tag="w",
                             name=f"w_{hb}_{wck}_{i}_{j}")
            nc.scalar.activation(w_t[:], s[:], mybir.ActivationFunctionType.Exp)
            nc.vector.tensor_add(wsum[:], wsum[:], w_t[:])
            t3 = spool.tile([128, WC * CO], FP32, tag="t3",
                            name=f"t3_{hb}_{wck}_{i}_{j}")
            w_b = w_t[:].unsqueeze(2).broadcast_to([128, WC, CO])
            f3 = _ilv(Fi[:, j * CF : (j + WC) * CF], WC)[:, :, 0:CO]
            nc.vector.tensor_mul(_ilv(t3[:], WC, CO), w_b, f3)
            nc.vector.tensor_add(acc[:], acc[:], t3[:])

    # ---- out = acc / wsum ----
    rec = spool.tile([128, WC], FP32, tag="s", name=f"rec_{hb}_{wck}")
    nc.vector.reciprocal(rec[:], wsum[:])
    out3 = spool.tile([128, WC * CO], FP32, tag="t3", name=f"out3_{hb}_{wck}")
    rec_b = rec[:].unsqueeze(2).broadcast_to([128, WC, CO])
    nc.vector.tensor_mul(_ilv(out3[:], WC, CO), rec_b, _ilv(acc[:], WC, CO))
    for c in range(CO):
        oc = spool.tile([128, WC], FP32, tag="oc", name=f"oc_{hb}_{wck}_{c}")
        nc.scalar.copy(oc[:], _ilv(out3[:], WC, CO)[:, :, c])
        nc.sync.dma_start(out=y[c, r0 : r0 + 128, w0 : w0 + WC], in_=oc[:])


def shard_inputs(input):
    """input [2,18,1024,1024] -> 8 per-core slabs [18, 262, 1024]."""
    input = np.asarray(input, dtype=np.float32)
    per_b = 4
    rows = H // per_b
    in_maps = []
    for core in range(8):
        b, q = divmod(core, per_b)
        r0 = q * rows
        slab = np.full((C_ALL, HIN, W), SENT, dtype=np.float32)
        s_lo = max(r0 - RAD, 0)
        s_hi = min(r0 + rows + RAD, H)
        slab[:, s_lo - (r0 - RAD) : s_hi - (r0 - RAD), :] = input[b, :, s_lo:s_hi, :]
        in_maps.append({"x": np.ascontiguousarray(slab)})
    return in_maps


def assemble(results):
    out = np.empty((B, CO, H, W), dtype=np.float32)
    rows = H // 4
    for core in range(8):
        b, q = divmod(core, 4)
        out[b, :, q * rows : (q + 1) * rows, :] = results[core]["y"]
    return out


def kernel(input):
    from concourse.bass_utils import run_bass_kernel_spmd

    if "nc" not in _CACHED:
        _CACHED["nc"] = build_nc()
    in_maps = shard_inputs(input)
    res = run_bass_kernel_spmd(_CACHED["nc"], in_maps, list(range(8)))
    return assemble(res.results)



# revision 4
# speedup vs baseline: 1.2877x; 1.2877x over previous
"""Bilateral filter (7x7, dilation 1) Trainium2 Bass kernel — v2.

Problem: input [2, 18, 1024, 1024] f32.
  filterable = input[:, :8]; params = input[:, 8:]
  r_c = -(p_c^2), sx = -(p8^2), sy = -(p9^2)
  logw = sum_c r_c (fn_c - f_c)^2 + sx dx^2 + sy dy^2  (OOB taps masked)
  out[c] = sum_taps w * fn_c / sum_taps w,  c < 3

Sharding: data-parallel over (batch, H): 8 cores, each 256 rows of one batch
image (+3 halo rows each side, sentinel-padded host-side, sentinel=100).

v2 design (per core, 2 row-blocks x 2 W-chunks of [128 rows, 512 cols]):
  - fp16 on-chip compute (DVE 2x_1P mode for all tensor_tensor ops),
    channel-planar ("c-major") free-axis layout [128, 8ch * 518cols] so every
    hot AP is unit-stride.
  - GPSIMD cast-DMA (f32->f16) loads a 134-row staging set; 7 row-shifted
    tile copies made with partition-shifted SBUF->SBUF DMAs on the SP queue.
  - Per tap: DVE sub; ACT Square (in-place); mult by p^2 split DVE/GPSIMD;
    pairwise-tree channel reduce on DVE (contiguous halves, 2x mode);
    + spatial term; clamp; ACT exp(scale=-1); w*fn3 (DVE/GPSIMD alternating);
    PE identity-matmul accumulates [w*fn3 | w] into PSUM across all 49 taps
    (fp32 accumulation for free on the otherwise idle tensor engine).
  - Weight math: w = exp(-(sum_c p_c^2 d_c^2 + a*sx^2 + b*sy^2)), all terms
    computed as positives.  Sentinel pixels drive the quadratic form to
    +huge (or +inf) -> exp -> +0, reproducing the reference's OOB mask.
"""

import sys

if "/opt/trn_rl_repo" not in sys.path:
    sys.path.insert(0, "/opt/trn_rl_repo")

import numpy as np

import concourse.bass as bass
import concourse.mybir as mybir
from concourse.bacc import Bacc
from concourse.tile import TileContext
from concourse.masks import make_identity

FP32 = mybir.dt.float32
FP16 = mybir.dt.float16
AF = mybir.ActivationFunctionType

B, C_ALL, H, W = 2, 18, 1024, 1024
CF = 8                      # filterable channels
CO = 3                      # output channels
KS, RAD = 7, 3
HC = H * B // 8             # 256 output rows per core
HIN = HC + 2 * RAD          # 262 input rows per core (halo padded host-side)
WC = 512                    # W chunk
NW = W // WC                # 2
NHB = HC // 128             # 2
WT = WC + 2 * RAD           # 518 (with column halo)
SENT = 8.0                  # sentinel: max quadratic form ~41K < fp16 max,
                            # so no inf on-chip, yet exp(-s) underflows to +0
D2IDX = [3, 2, 1, 0, 1, 2, 3]              # index into D2VALS: (k-3)^2
D2VALS = [0.0, 1.0, 4.0, 9.0]

# engine-split tuning knobs
MD = 4          # channels of the p^2*d^2 multiply done on DVE (rest GPSIMD)
GP_MUL3 = 2     # every GP_MUL3-th tap's w*fn3 runs on GPSIMD (0 = never)

_CACHED = {}


def _cm(ap, w=WT, c=CF):
    """View flat [128, c*w] as [128, c, w] (channel-major blocks)."""
    return ap.rearrange("p (c x) -> p c x", c=c)


def build_nc():
    nc = Bacc()
    x = nc.dram_tensor("x", [HIN, C_ALL, W], FP32, kind="ExternalInput")
    y = nc.dram_tensor("y", [CO, HC, W], FP32, kind="ExternalOutput")

    with TileContext(nc) as tc:
        with (
            tc.tile_pool(name="ipool", bufs=1) as ipool,
            tc.tile_pool(name="fpool", bufs=1) as fpool,
            tc.tile_pool(name="cpool", bufs=1) as cpool,
            tc.tile_pool(name="dpool", bufs=1) as dpool,
            tc.tile_pool(name="spool", bufs=1) as spool,
            tc.tile_pool(name="ppool", bufs=1, space="PSUM") as ppool,
        ):
            ident = ipool.tile([128, 128], FP16, tag="ident", name="ident")
            make_identity(nc, ident[:])
            for hb in range(NHB):
                for wck in range(NW):
                    _macro(nc, tc, x, y, ident, fpool, cpool, dpool, spool,
                           ppool, hb, wck)
    nc.compile()
    return nc


def _macro(nc, tc, x, y, ident, fpool, cpool, dpool, spool, ppool, hb, wck):
    w0 = wck * WC
    r0 = hb * 128
    # staging-tile col t <-> image col w0 - 3 + t
    lo = RAD if wck == 0 else 0
    hi = WT - RAD if wck == NW - 1 else WT

    # ---- staging: cast-DMA f32 -> f16, rows r0 .. r0+133 of the slab ----
    # Ct: slab rows r0..r0+2 | Fm: r0+3..r0+130 (= center tile F[3]) |
    # Cb: r0+131..r0+133
    Ct = fpool.tile([RAD, CF * WT], FP16, tag="Ct", name=f"Ct_{hb}_{wck}")
    Fm = fpool.tile([128, CF * WT], FP16, tag="Fm", name=f"Fm_{hb}_{wck}")
    Cb = fpool.tile([RAD, CF * WT], FP16, tag="Cb", name=f"Cb_{hb}_{wck}")

    for tile, rb, nr in ((Ct, r0, RAD), (Fm, r0 + RAD, 128),
                         (Cb, r0 + RAD + 128, RAD)):
        v = tile[:].rearrange("p (c x) -> p c x", c=CF)
        if lo > 0:
            nc.gpsimd.memset(v[:, :, 0:lo], SENT)
        if hi < WT:
            nc.gpsimd.memset(v[:, :, hi:WT], SENT)
        nc.gpsimd.dma_start(
            out=v[:, :, lo:hi],
            in_=x[rb : rb + nr, 0:CF, w0 - RAD + lo : w0 - RAD + hi],
        )

    # ---- 7 row-shifted tiles: F[oy][p] = staging row oy+p ----
    F = [None] * KS
    F[RAD] = Fm
    for oy in range(KS):
        if oy == RAD:
            continue
        Ft = fpool.tile([128, CF * WT], FP16, tag=f"F{oy}",
                        name=f"F{oy}_{hb}_{wck}")
        if oy < RAD:
            k = RAD - oy  # rows from Ct
            nc.sync.dma_start(out=Ft[0:k, :], in_=Ct[oy:RAD, :])
            nc.sync.dma_start(out=Ft[k:128, :], in_=Fm[0 : 128 - k, :])
        else:
            k = oy - RAD  # rows from Cb
            nc.sync.dma_start(out=Ft[0 : 128 - k, :], in_=Fm[k:128, :])
            nc.sync.dma_start(out=Ft[128 - k : 128, :], in_=Cb[0:k, :])
        F[oy] = Ft
    Fc = _cm(Fm[:])[:, :, RAD : RAD + WC]

    # ---- params: P2[c] = p_c^2 (f16, c-major), sx2/sy2 ----
    P2 = cpool.tile([128, CF * WC], FP16, tag="P2", name=f"P2_{hb}_{wck}")
    sxy2 = cpool.tile([128, 2 * WC], FP16, tag="sxy2", name=f"sxy2_{hb}_{wck}")
    for k in range(CF + 2):
        pst = fpool.tile([128, WC], FP32, tag="pst", bufs=2,
                         name=f"pst_{hb}_{wck}_{k}")
        nc.sync.dma_start(
            out=pst[:],
            in_=x[r0 + RAD : r0 + RAD + 128, CF + k, w0 : w0 + WC])
        dst = (P2[:, k * WC : (k + 1) * WC] if k < CF
               else sxy2[:, (k - CF) * WC : (k - CF + 1) * WC])
        nc.scalar.activation(dst, pst[:], AF.Square)
    sx2 = sxy2[:, 0:WC]
    sy2 = sxy2[:, WC : 2 * WC]

    # ---- spatial log-weights: asp(a, b) = a*sx2 + b*sy2 (positive) ----
    Aa = cpool.tile([128, 3 * WC], FP16, tag="Aa", name=f"Aa_{hb}_{wck}")
    Ab = cpool.tile([128, 3 * WC], FP16, tag="Ab", name=f"Ab_{hb}_{wck}")
    for ai in (1, 2, 3):
        nc.vector.tensor_scalar_mul(
            Aa[:, (ai - 1) * WC : ai * WC], sx2, float(D2VALS[ai]))
        nc.vector.tensor_scalar_mul(
            Ab[:, (ai - 1) * WC : ai * WC], sy2, float(D2VALS[ai]))
    Asum = cpool.tile([128, 9 * WC], FP16, tag="Asum", name=f"As_{hb}_{wck}")
    for ai in (1, 2, 3):
        for bi in (1, 2, 3):
            k = (ai - 1) * 3 + (bi - 1)
            nc.vector.tensor_add(
                Asum[:, k * WC : (k + 1) * WC],
                Aa[:, (ai - 1) * WC : ai * WC],
                Ab[:, (bi - 1) * WC : bi * WC])

    def asp_ap(i, j):
        ai, bi = D2IDX[j], D2IDX[i]   # x-dist from col shift j, y from row i
        if ai == 0 and bi == 0:
            return None
        if bi == 0:
            return Aa[:, (ai - 1) * WC : ai * WC]
        if ai == 0:
            return Ab[:, (bi - 1) * WC : bi * WC]
        k = (ai - 1) * 3 + (bi - 1)
        return Asum[:, k * WC : (k + 1) * WC]

    # ---- PSUM accumulator: [w*fn0 | w*fn1 | w*fn2 | w] ----
    ps = ppool.tile([128, 4 * WC], FP32, tag="ps", bufs=2,
                    name=f"ps_{hb}_{wck}")

    taps = [(i, j) for i in range(KS) for j in range(KS)]
    n = len(taps)
    Dt, Tt = {}, {}

    def stage_a(t):     # sub + square (in-place)
        i, j = taps[t]
        d = dpool.tile([128, CF * WC], FP16, tag="d", bufs=3,
                       name=f"d_{hb}_{wck}_{t}")
        nc.vector.tensor_sub(_cm(d[:], WC), _cm(F[i][:])[:, :, j : j + WC], Fc)
        nc.scalar.activation(d[:], d[:], AF.Square)
        Dt[t] = d

    def stage_b(t):     # p^2 multiply, tree-reduce, +asp, clamp, exp
        i, j = taps[t]
        dv = Dt.pop(t)[:]
        if MD > 0:
            nc.vector.tensor_mul(dv[:, 0 : MD * WC], P2[:, 0 : MD * WC],
                                 dv[:, 0 : MD * WC])
        if MD < CF:
            nc.gpsimd.tensor_mul(dv[:, MD * WC :], P2[:, MD * WC :],
                                 dv[:, MD * WC :])
        nc.vector.tensor_add(dv[:, 0 : 4 * WC], dv[:, 0 : 4 * WC],
                             dv[:, 4 * WC : 8 * WC])
        nc.vector.tensor_add(dv[:, 0 : 2 * WC], dv[:, 0 : 2 * WC],
                             dv[:, 2 * WC : 4 * WC])
        nc.vector.tensor_add(dv[:, 0:WC], dv[:, 0:WC], dv[:, WC : 2 * WC])
        ap = asp_ap(i, j)
        if ap is not None:
            nc.vector.tensor_add(dv[:, 0:WC], dv[:, 0:WC], ap)
        T = spool.tile([128, 4 * WC], FP16, tag="T", bufs=3,
                       name=f"T_{hb}_{wck}_{t}")
        nc.scalar.activation(T[:, 3 * WC : 4 * WC], dv[:, 0:WC], AF.Exp,
                             scale=-1.0)
        Tt[t] = T

    def stage_c(t):     # w*fn3, then PE accumulates [w*fn3 | w] into PSUM
        i, j = taps[t]
        T = Tt.pop(t)
        w_b = T[:, 3 * WC : 4 * WC].unsqueeze(1).broadcast_to([128, CO, WC])
        fn3 = _cm(F[i][:])[:, 0:CO, j : j + WC]
        eng = nc.gpsimd if (GP_MUL3 and t % GP_MUL3 == 0) else nc.vector
        eng.tensor_mul(_cm(T[:, 0 : CO * WC], WC, CO), w_b, fn3)
        for k in range(4):
            nc.tensor.matmul(
                ps[:, k * WC : (k + 1) * WC], ident[:],
                T[:, k * WC : (k + 1) * WC],
                start=(t == 0), stop=(t == n - 1))

    for t in range(n + 2):
        if t < n:
            stage_a(t)
        if 0 <= t - 1 < n:
            stage_b(t - 1)
        if 0 <= t - 2 < n:
            stage_c(t - 2)

    # ---- out = acc / wsum ----
    rec = spool.tile([128, WC], FP32, tag="rec", name=f"rec_{hb}_{wck}")
    nc.vector.reciprocal(rec[:], ps[:, 3 * WC : 4 * WC])
    out3 = spool.tile([128, CO * WC], FP32, tag="out3", name=f"o3_{hb}_{wck}")
    rec_b = rec[:].unsqueeze(1).broadcast_to([128, CO, WC])
    nc.vector.tensor_mul(_cm(out3[:], WC, CO), rec_b,
                         _cm(ps[:, 0 : CO * WC], WC, CO))
    for c in range(CO):
        nc.sync.dma_start(out=y[c, r0 : r0 + 128, w0 : w0 + WC],
                          in_=out3[:, c * WC : (c + 1) * WC])


def shard_inputs(input):
    """input [2,18,1024,1024] -> 8 per-core slabs [262, 18, 1024]."""
    input = np.asarray(input, dtype=np.float32)
    per_b = 4
    rows = H // per_b
    in_maps = []
    for core in range(8):
        b, q = divmod(core, per_b)
        r0 = q * rows
        slab = np.full((HIN, C_ALL, W), SENT, dtype=np.float32)
        s_lo = max(r0 - RAD, 0)
        s_hi = min(r0 + rows + RAD, H)
        slab[s_lo - (r0 - RAD) : s_hi - (r0 - RAD), :, :] = (
            input[b, :, s_lo:s_hi, :].transpose(1, 0, 2))
        in_maps.append({"x": np.ascontiguousarray(slab)})
    return in_maps


def assemble(results):
    out = np.empty((B, CO, H, W), dtype=np.float32)
    rows = H // 4
    for core in range(8):
        b, q = divmod(core, 4)
        out[b, :, q * rows : (q + 1) * rows, :] = results[core]["y"]
    return out


def kernel(input):
    from concourse.bass_utils import run_bass_kernel_spmd

    if "nc" not in _CACHED:
        _CACHED["nc"] = build_nc()
    in_maps = shard_inputs(input)
    res = run_bass_kernel_spmd(_CACHED["nc"], in_maps, list(range(8)))
    return assemble(res.results)


# revision 6
# speedup vs baseline: 1.3851x; 1.0757x over previous
"""Bilateral filter (7x7, dilation 1) Trainium2 Bass kernel — v2.

Problem: input [2, 18, 1024, 1024] f32.
  filterable = input[:, :8]; params = input[:, 8:]
  r_c = -(p_c^2), sx = -(p8^2), sy = -(p9^2)
  logw = sum_c r_c (fn_c - f_c)^2 + sx dx^2 + sy dy^2  (OOB taps masked)
  out[c] = sum_taps w * fn_c / sum_taps w,  c < 3

Sharding: data-parallel over (batch, H): 8 cores, each 256 rows of one batch
image (+3 halo rows each side, sentinel-padded host-side, sentinel=100).

v2 design (per core, 2 row-blocks x 2 W-chunks of [128 rows, 512 cols]):
  - fp16 on-chip compute (DVE 2x_1P mode for all tensor_tensor ops),
    channel-planar ("c-major") free-axis layout [128, 8ch * 518cols] so every
    hot AP is unit-stride.
  - GPSIMD cast-DMA (f32->f16) loads a 134-row staging set; 7 row-shifted
    tile copies made with partition-shifted SBUF->SBUF DMAs on the SP queue.
  - Per tap: DVE sub; ACT Square (in-place); mult by p^2 split DVE/GPSIMD;
    pairwise-tree channel reduce on DVE (contiguous halves, 2x mode);
    + spatial term; clamp; ACT exp(scale=-1); w*fn3 (DVE/GPSIMD alternating);
    PE identity-matmul accumulates [w*fn3 | w] into PSUM across all 49 taps
    (fp32 accumulation for free on the otherwise idle tensor engine).
  - Weight math: w = exp(-(sum_c p_c^2 d_c^2 + a*sx^2 + b*sy^2)), all terms
    computed as positives.  Sentinel pixels drive the quadratic form to
    +huge (or +inf) -> exp -> +0, reproducing the reference's OOB mask.
"""

import sys

if "/opt/trn_rl_repo" not in sys.path:
    sys.path.insert(0, "/opt/trn_rl_repo")

import numpy as np

import concourse.bass as bass
import concourse.mybir as mybir
from concourse.bacc import Bacc
from concourse.tile import TileContext
from concourse.masks import make_identity

FP32 = mybir.dt.float32
FP16 = mybir.dt.float16
AF = mybir.ActivationFunctionType

B, C_ALL, H, W = 2, 18, 1024, 1024
CF = 8                      # filterable channels
CO = 3                      # output channels
KS, RAD = 7, 3
HC = H * B // 8             # 256 output rows per core
HIN = HC + 2 * RAD          # 262 input rows per core (halo padded host-side)
WC = 512                    # W chunk
NW = W // WC                # 2
NHB = HC // 128             # 2
WT = WC + 2 * RAD           # 518 (with column halo)
SENT = 8.0                  # sentinel: max quadratic form ~41K < fp16 max,
                            # so no inf on-chip, yet exp(-s) underflows to +0
D2IDX = [3, 2, 1, 0, 1, 2, 3]              # index into D2VALS: (k-3)^2
D2VALS = [0.0, 1.0, 4.0, 9.0]

# engine-split tuning knobs
MD = 6          # channels of the p^2*d^2 multiply done on DVE (rest GPSIMD)
GP_MUL3 = 1     # every GP_MUL3-th tap's w*fn3 runs on GPSIMD (0 = never)
CTR = KS // 2 * KS + KS // 2   # center tap index (w == 1 fast path)

_CACHED = {}


def _cm(ap, w=WT, c=CF):
    """View flat [128, c*w] as [128, c, w] (channel-major blocks)."""
    return ap.rearrange("p (c x) -> p c x", c=c)


def build_nc():
    nc = Bacc()
    x = nc.dram_tensor("x", [HIN, C_ALL, W], FP32, kind="ExternalInput")
    y = nc.dram_tensor("y", [CO, HC, W], FP32, kind="ExternalOutput")

    with TileContext(nc) as tc:
        with (
            tc.tile_pool(name="ipool", bufs=1) as ipool,
            tc.tile_pool(name="fpool", bufs=1) as fpool,
            tc.tile_pool(name="cpool", bufs=1) as cpool,
            tc.tile_pool(name="dpool", bufs=1) as dpool,
            tc.tile_pool(name="spool", bufs=1) as spool,
            tc.tile_pool(name="ppool", bufs=1, space="PSUM") as ppool,
        ):
            ident = ipool.tile([128, 128], FP16, tag="ident", name="ident")
            make_identity(nc, ident[:])
            for hb in range(NHB):
                for wck in range(NW):
                    _macro(nc, tc, x, y, ident, fpool, cpool, dpool, spool,
                           ppool, hb, wck)
    nc.compile()
    return nc


def _macro(nc, tc, x, y, ident, fpool, cpool, dpool, spool, ppool, hb, wck):
    w0 = wck * WC
    r0 = hb * 128
    # staging-tile col t <-> image col w0 - 3 + t
    lo = RAD if wck == 0 else 0
    hi = WT - RAD if wck == NW - 1 else WT

    # ---- staging: cast-DMA f32 -> f16, rows r0 .. r0+133 of the slab ----
    # Ct: slab rows r0..r0+2 | Fm: r0+3..r0+130 (= center tile F[3]) |
    # Cb: r0+131..r0+133
    Ct = fpool.tile([RAD, CF * WT], FP16, tag="Ct", name=f"Ct_{hb}_{wck}")
    Fm = fpool.tile([128, CF * WT], FP16, tag="Fm", name=f"Fm_{hb}_{wck}")
    Cb = fpool.tile([RAD, CF * WT], FP16, tag="Cb", name=f"Cb_{hb}_{wck}")

    for tile, rb, nr in ((Ct, r0, RAD), (Fm, r0 + RAD, 128),
                         (Cb, r0 + RAD + 128, RAD)):
        v = tile[:].rearrange("p (c x) -> p c x", c=CF)
        if lo > 0:
            nc.gpsimd.memset(v[:, :, 0:lo], SENT)
        if hi < WT:
            nc.gpsimd.memset(v[:, :, hi:WT], SENT)
        nc.gpsimd.dma_start(
            out=v[:, :, lo:hi],
            in_=x[rb : rb + nr, 0:CF, w0 - RAD + lo : w0 - RAD + hi],
        )

    # ---- 7 row-shifted tiles: F[oy][p] = staging row oy+p ----
    F = [None] * KS
    F[RAD] = Fm
    for oy in range(KS):
        if oy == RAD:
            continue
        Ft = fpool.tile([128, CF * WT], FP16, tag=f"F{oy}",
                        name=f"F{oy}_{hb}_{wck}")
        if oy < RAD:
            k = RAD - oy  # rows from Ct
            nc.sync.dma_start(out=Ft[0:k, :], in_=Ct[oy:RAD, :])
            nc.sync.dma_start(out=Ft[k:128, :], in_=Fm[0 : 128 - k, :])
        else:
            k = oy - RAD  # rows from Cb
            nc.sync.dma_start(out=Ft[0 : 128 - k, :], in_=Fm[k:128, :])
            nc.sync.dma_start(out=Ft[128 - k : 128, :], in_=Cb[0:k, :])
        F[oy] = Ft
    Fc = _cm(Fm[:])[:, :, RAD : RAD + WC]

    # ---- params: P2[c] = p_c^2 (f16, c-major), sx2/sy2 ----
    P2 = cpool.tile([128, CF * WC], FP16, tag="P2", name=f"P2_{hb}_{wck}")
    sxy2 = cpool.tile([128, 2 * WC], FP16, tag="sxy2", name=f"sxy2_{hb}_{wck}")
    for k in range(CF + 2):
        pst = fpool.tile([128, WC], FP32, tag="pst", bufs=2,
                         name=f"pst_{hb}_{wck}_{k}")
        nc.sync.dma_start(
            out=pst[:],
            in_=x[r0 + RAD : r0 + RAD + 128, CF + k, w0 : w0 + WC])
        dst = (P2[:, k * WC : (k + 1) * WC] if k < CF
               else sxy2[:, (k - CF) * WC : (k - CF + 1) * WC])
        nc.scalar.activation(dst, pst[:], AF.Square)
    sx2 = sxy2[:, 0:WC]
    sy2 = sxy2[:, WC : 2 * WC]

    # ---- spatial log-weights: asp(a, b) = a*sx2 + b*sy2 (positive) ----
    Aa = cpool.tile([128, 3 * WC], FP16, tag="Aa", name=f"Aa_{hb}_{wck}")
    Ab = cpool.tile([128, 3 * WC], FP16, tag="Ab", name=f"Ab_{hb}_{wck}")
    for ai in (1, 2, 3):
        nc.vector.tensor_scalar_mul(
            Aa[:, (ai - 1) * WC : ai * WC], sx2, float(D2VALS[ai]))
        nc.vector.tensor_scalar_mul(
            Ab[:, (ai - 1) * WC : ai * WC], sy2, float(D2VALS[ai]))
    Asum = cpool.tile([128, 9 * WC], FP16, tag="Asum", name=f"As_{hb}_{wck}")
    for ai in (1, 2, 3):
        for bi in (1, 2, 3):
            k = (ai - 1) * 3 + (bi - 1)
            nc.vector.tensor_add(
                Asum[:, k * WC : (k + 1) * WC],
                Aa[:, (ai - 1) * WC : ai * WC],
                Ab[:, (bi - 1) * WC : bi * WC])

    def asp_ap(i, j):
        ai, bi = D2IDX[j], D2IDX[i]   # x-dist from col shift j, y from row i
        if ai == 0 and bi == 0:
            return None
        if bi == 0:
            return Aa[:, (ai - 1) * WC : ai * WC]
        if ai == 0:
            return Ab[:, (bi - 1) * WC : bi * WC]
        k = (ai - 1) * 3 + (bi - 1)
        return Asum[:, k * WC : (k + 1) * WC]

    # ---- PSUM accumulator: [w*fn0 | w*fn1 | w*fn2 | w] ----
    ps = ppool.tile([128, 4 * WC], FP32, tag="ps", bufs=2,
                    name=f"ps_{hb}_{wck}")

    taps = [(i, j) for i in range(KS) for j in range(KS)]
    n = len(taps)
    Dt, Tt = {}, {}

    def stage_a(t):     # sub + square (in-place)
        if t == CTR:
            return
        i, j = taps[t]
        d = dpool.tile([128, CF * WC], FP16, tag="d", bufs=4,
                       name=f"d_{hb}_{wck}_{t}")
        nc.vector.tensor_sub(_cm(d[:], WC), _cm(F[i][:])[:, :, j : j + WC], Fc)
        nc.scalar.activation(d[:], d[:], AF.Square)
        Dt[t] = d

    def stage_m(t):     # p^2 multiply, split DVE / GPSIMD
        if t == CTR:
            return
        dv = Dt[t][:]
        if MD > 0:
            nc.vector.tensor_mul(dv[:, 0 : MD * WC], P2[:, 0 : MD * WC],
                                 dv[:, 0 : MD * WC])
        if MD < CF:
            nc.gpsimd.tensor_mul(dv[:, MD * WC :], P2[:, MD * WC :],
                                 dv[:, MD * WC :])

    def stage_r(t):     # tree-reduce, +asp, exp
        T = spool.tile([128, 4 * WC], FP16, tag="T", bufs=4,
                       name=f"T_{hb}_{wck}_{t}")
        Tt[t] = T
        if t == CTR:
            nc.gpsimd.memset(T[:, 3 * WC : 4 * WC], 1.0)
            return
        i, j = taps[t]
        dv = Dt.pop(t)[:]
        nc.vector.tensor_add(dv[:, 0 : 4 * WC], dv[:, 0 : 4 * WC],
                             dv[:, 4 * WC : 8 * WC])
        nc.vector.tensor_add(dv[:, 0 : 2 * WC], dv[:, 0 : 2 * WC],
                             dv[:, 2 * WC : 4 * WC])
        nc.vector.tensor_add(dv[:, 0:WC], dv[:, 0:WC], dv[:, WC : 2 * WC])
        ap = asp_ap(i, j)
        if ap is not None:
            nc.vector.tensor_add(dv[:, 0:WC], dv[:, 0:WC], ap)
        nc.scalar.activation(T[:, 3 * WC : 4 * WC], dv[:, 0:WC], AF.Exp,
                             scale=-1.0)

    def stage_c(t):     # w*fn3, then PE accumulates [w*fn3 | w] into PSUM
        i, j = taps[t]
        T = Tt.pop(t)
        fn3 = _cm(F[i][:])[:, 0:CO, j : j + WC]
        if t == CTR:
            nc.vector.tensor_copy(_cm(T[:, 0 : CO * WC], WC, CO), fn3)
        else:
            w_b = T[:, 3 * WC : 4 * WC].unsqueeze(1).broadcast_to(
                [128, CO, WC])
            eng = nc.gpsimd if (GP_MUL3 and t % GP_MUL3 == 0) else nc.vector
            eng.tensor_mul(_cm(T[:, 0 : CO * WC], WC, CO), w_b, fn3)
        for k in range(4):
            nc.tensor.matmul(
                ps[:, k * WC : (k + 1) * WC], ident[:],
                T[:, k * WC : (k + 1) * WC],
                start=(t == 0), stop=(t == n - 1))

    for t in range(n + 3):
        if t < n:
            stage_a(t)
        if 0 <= t - 1 < n:
            stage_m(t - 1)
        if 0 <= t - 2 < n:
            stage_r(t - 2)
        if 0 <= t - 3 < n:
            stage_c(t - 3)

    # ---- out = acc / wsum ----
    rec = spool.tile([128, WC], FP32, tag="rec", name=f"rec_{hb}_{wck}")
    nc.vector.reciprocal(rec[:], ps[:, 3 * WC : 4 * WC])
    out3 = spool.tile([128, CO * WC], FP32, tag="out3", name=f"o3_{hb}_{wck}")
    rec_b = rec[:].unsqueeze(1).broadcast_to([128, CO, WC])
    nc.vector.tensor_mul(_cm(out3[:], WC, CO), rec_b,
                         _cm(ps[:, 0 : CO * WC], WC, CO))
    for c in range(CO):
        nc.sync.dma_start(out=y[c, r0 : r0 + 128, w0 : w0 + WC],
                          in_=out3[:, c * WC : (c + 1) * WC])


def shard_inputs(input):
    """input [2,18,1024,1024] -> 8 per-core slabs [262, 18, 1024]."""
    input = np.asarray(input, dtype=np.float32)
    per_b = 4
    rows = H // per_b
    in_maps = []
    for core in range(8):
        b, q = divmod(core, per_b)
        r0 = q * rows
        slab = np.full((HIN, C_ALL, W), SENT, dtype=np.float32)
        s_lo = max(r0 - RAD, 0)
        s_hi = min(r0 + rows + RAD, H)
        slab[s_lo - (r0 - RAD) : s_hi - (r0 - RAD), :, :] = (
            input[b, :, s_lo:s_hi, :].transpose(1, 0, 2))
        in_maps.append({"x": np.ascontiguousarray(slab)})
    return in_maps


def assemble(results):
    out = np.empty((B, CO, H, W), dtype=np.float32)
    rows = H // 4
    for core in range(8):
        b, q = divmod(core, 4)
        out[b, :, q * rows : (q + 1) * rows, :] = results[core]["y"]
    return out


def kernel(input):
    from concourse.bass_utils import run_bass_kernel_spmd

    if "nc" not in _CACHED:
        _CACHED["nc"] = build_nc()
    in_maps = shard_inputs(input)
    res = run_bass_kernel_spmd(_CACHED["nc"], in_maps, list(range(8)))
    return assemble(res.results)


# revision 9
# speedup vs baseline: 1.6113x; 1.1633x over previous
"""Bilateral filter (7x7, dilation 1) Trainium2 Bass kernel — v2.

Problem: input [2, 18, 1024, 1024] f32.
  filterable = input[:, :8]; params = input[:, 8:]
  r_c = -(p_c^2), sx = -(p8^2), sy = -(p9^2)
  logw = sum_c r_c (fn_c - f_c)^2 + sx dx^2 + sy dy^2  (OOB taps masked)
  out[c] = sum_taps w * fn_c / sum_taps w,  c < 3

Sharding: data-parallel over (batch, H): 8 cores, each 256 rows of one batch
image (+3 halo rows each side, sentinel-padded host-side, sentinel=100).

v2 design (per core, 2 row-blocks x 2 W-chunks of [128 rows, 512 cols]):
  - fp16 on-chip compute (DVE 2x_1P mode for all tensor_tensor ops),
    channel-planar ("c-major") free-axis layout [128, 8ch * 518cols] so every
    hot AP is unit-stride.
  - GPSIMD cast-DMA (f32->f16) loads a 134-row staging set; 7 row-shifted
    tile copies made with partition-shifted SBUF->SBUF DMAs on the SP queue.
  - Per tap: DVE sub; ACT Square (in-place); mult by p^2 split DVE/GPSIMD;
    pairwise-tree channel reduce on DVE (contiguous halves, 2x mode);
    + spatial term; clamp; ACT exp(scale=-1); w*fn3 (DVE/GPSIMD alternating);
    PE identity-matmul accumulates [w*fn3 | w] into PSUM across all 49 taps
    (fp32 accumulation for free on the otherwise idle tensor engine).
  - Weight math: w = exp(-(sum_c p_c^2 d_c^2 + a*sx^2 + b*sy^2)), all terms
    computed as positives.  Sentinel pixels drive the quadratic form to
    +huge (or +inf) -> exp -> +0, reproducing the reference's OOB mask.
"""

import sys

if "/opt/trn_rl_repo" not in sys.path:
    sys.path.insert(0, "/opt/trn_rl_repo")

import numpy as np

import concourse.bass as bass
import concourse.mybir as mybir
from concourse.bacc import Bacc
from concourse.tile import TileContext
from concourse.masks import make_identity

FP32 = mybir.dt.float32
FP16 = mybir.dt.float16
AF = mybir.ActivationFunctionType

B, C_ALL, H, W = 2, 18, 1024, 1024
CF = 8                      # filterable channels
CO = 3                      # output channels
KS, RAD = 7, 3
HC = H * B // 8             # 256 output rows per core
HIN = HC + 2 * RAD          # 262 input rows per core (halo padded host-side)
WC = 512                    # W chunk
NW = W // WC                # 2
NHB = HC // 128             # 2
WT = WC + 2 * RAD           # 518 (with column halo)
SENT = 8.0                  # sentinel: max quadratic form ~41K < fp16 max,
                            # so no inf on-chip, yet exp(-s) underflows to +0
D2IDX = [3, 2, 1, 0, 1, 2, 3]              # index into D2VALS: (k-3)^2
D2VALS = [0.0, 1.0, 4.0, 9.0]

# engine-split tuning knobs
MD = 6          # channels of the p^2*d^2 multiply done on DVE (rest GPSIMD)
GP_MUL3 = 1     # every GP_MUL3-th tap's w*fn3 runs on GPSIMD (0 = never)
CTR = KS // 2 * KS + KS // 2   # center tap index (w == 1 fast path)

_CACHED = {}


def _cm(ap, w=WT, c=CF):
    """View flat [128, c*w] as [128, c, w] (channel-major blocks)."""
    return ap.rearrange("p (c x) -> p c x", c=c)


def build_nc():
    nc = Bacc()
    x = nc.dram_tensor("x", [HIN, C_ALL, W], FP32, kind="ExternalInput")
    y = nc.dram_tensor("y", [CO, HC, W], FP32, kind="ExternalOutput")

    with TileContext(nc) as tc:
        with (
            tc.tile_pool(name="ipool", bufs=1) as ipool,
            tc.tile_pool(name="fpool", bufs=1) as fpool,
            tc.tile_pool(name="cpool", bufs=1) as cpool,
            tc.tile_pool(name="dpool", bufs=1) as dpool,
            tc.tile_pool(name="spool", bufs=1) as spool,
            tc.tile_pool(name="ppool", bufs=1, space="PSUM") as ppool,
        ):
            ident = ipool.tile([128, 128], FP16, tag="ident", name="ident")
            make_identity(nc, ident[:])
            for hb in range(NHB):
                for wck in range(NW):
                    _macro(nc, tc, x, y, ident, fpool, cpool, dpool, spool,
                           ppool, hb, wck)
    nc.compile()
    return nc


def _macro(nc, tc, x, y, ident, fpool, cpool, dpool, spool, ppool, hb, wck):
    w0 = wck * WC
    r0 = hb * 128
    # staging-tile col t <-> image col w0 - 3 + t
    lo = RAD if wck == 0 else 0
    hi = WT - RAD if wck == NW - 1 else WT

    # ---- staging: cast-DMA f32 -> f16, rows r0 .. r0+133 of the slab ----
    # Ct: slab rows r0..r0+2 | Fm: r0+3..r0+130 (= center tile F[3]) |
    # Cb: r0+131..r0+133
    Ct = fpool.tile([RAD, CF * WT], FP16, tag="Ct", name=f"Ct_{hb}_{wck}")
    Fm = fpool.tile([128, CF * WT], FP16, tag="Fm", name=f"Fm_{hb}_{wck}")
    Cb = fpool.tile([RAD, CF * WT], FP16, tag="Cb", name=f"Cb_{hb}_{wck}")

    for tile, rb, nr in ((Ct, r0, RAD), (Fm, r0 + RAD, 128),
                         (Cb, r0 + RAD + 128, RAD)):
        v = tile[:].rearrange("p (c x) -> p c x", c=CF)
        if lo > 0:
            nc.gpsimd.memset(v[:, :, 0:lo], SENT)
        if hi < WT:
            nc.gpsimd.memset(v[:, :, hi:WT], SENT)
        nc.gpsimd.dma_start(
            out=v[:, :, lo:hi],
            in_=x[rb : rb + nr, 0:CF, w0 - RAD + lo : w0 - RAD + hi],
        )

    # ---- 7 row-shifted tiles: F[oy][p] = staging row oy+p ----
    F = [None] * KS
    F[RAD] = Fm
    for oy in range(KS):
        if oy == RAD:
            continue
        Ft = fpool.tile([128, CF * WT], FP16, tag=f"F{oy}",
                        name=f"F{oy}_{hb}_{wck}")
        if oy < RAD:
            k = RAD - oy  # rows from Ct
            nc.sync.dma_start(out=Ft[0:k, :], in_=Ct[oy:RAD, :])
            nc.sync.dma_start(out=Ft[k:128, :], in_=Fm[0 : 128 - k, :])
        else:
            k = oy - RAD  # rows from Cb
            nc.sync.dma_start(out=Ft[0 : 128 - k, :], in_=Fm[k:128, :])
            nc.sync.dma_start(out=Ft[128 - k : 128, :], in_=Cb[0:k, :])
        F[oy] = Ft
    Fc = _cm(Fm[:])[:, :, RAD : RAD + WC]

    # ---- params: P2[c] = p_c^2 (f16, c-major), sx2/sy2 ----
    P2 = cpool.tile([128, CF * WC], FP16, tag="P2", name=f"P2_{hb}_{wck}")
    sxy2 = cpool.tile([128, 2 * WC], FP16, tag="sxy2", name=f"sxy2_{hb}_{wck}")
    for k in range(CF + 2):
        pst = fpool.tile([128, WC], FP32, tag="pst", bufs=2,
                         name=f"pst_{hb}_{wck}_{k}")
        nc.sync.dma_start(
            out=pst[:],
            in_=x[r0 + RAD : r0 + RAD + 128, CF + k, w0 : w0 + WC])
        dst = (P2[:, k * WC : (k + 1) * WC] if k < CF
               else sxy2[:, (k - CF) * WC : (k - CF + 1) * WC])
        nc.scalar.activation(dst, pst[:], AF.Square)
    sx2 = sxy2[:, 0:WC]
    sy2 = sxy2[:, WC : 2 * WC]

    # ---- spatial log-weights: asp(a, b) = a*sx2 + b*sy2 (positive) ----
    Aa = cpool.tile([128, 3 * WC], FP16, tag="Aa", name=f"Aa_{hb}_{wck}")
    Ab = cpool.tile([128, 3 * WC], FP16, tag="Ab", name=f"Ab_{hb}_{wck}")
    for ai in (1, 2, 3):
        nc.vector.tensor_scalar_mul(
            Aa[:, (ai - 1) * WC : ai * WC], sx2, float(D2VALS[ai]))
        nc.vector.tensor_scalar_mul(
            Ab[:, (ai - 1) * WC : ai * WC], sy2, float(D2VALS[ai]))
    Asum = cpool.tile([128, 9 * WC], FP16, tag="Asum", name=f"As_{hb}_{wck}")
    for ai in (1, 2, 3):
        for bi in (1, 2, 3):
            k = (ai - 1) * 3 + (bi - 1)
            nc.vector.tensor_add(
                Asum[:, k * WC : (k + 1) * WC],
                Aa[:, (ai - 1) * WC : ai * WC],
                Ab[:, (bi - 1) * WC : bi * WC])

    def asp_ap(i, j):
        ai, bi = D2IDX[j], D2IDX[i]   # x-dist from col shift j, y from row i
        if ai == 0 and bi == 0:
            return None
        if bi == 0:
            return Aa[:, (ai - 1) * WC : ai * WC]
        if ai == 0:
            return Ab[:, (bi - 1) * WC : bi * WC]
        k = (ai - 1) * 3 + (bi - 1)
        return Asum[:, k * WC : (k + 1) * WC]

    # ---- PSUM accumulator: [w*fn0 | w*fn1 | w*fn2 | w] ----
    ps = ppool.tile([128, 4 * WC], FP32, tag="ps", bufs=2,
                    name=f"ps_{hb}_{wck}")

    taps = [(i, j) for i in range(KS) for j in range(KS)]
    n = len(taps)
    Dt, Tt = {}, {}

    def stage_a(t):     # sub + square (in-place)
        if t == CTR:
            return
        i, j = taps[t]
        d = dpool.tile([128, CF * WC], FP16, tag="d", bufs=6,
                       name=f"d_{hb}_{wck}_{t}")
        nc.vector.tensor_sub(_cm(d[:], WC), _cm(F[i][:])[:, :, j : j + WC], Fc)
        nc.scalar.activation(d[:], d[:], AF.Square)
        Dt[t] = d

    def stage_m(t):     # p^2 multiply, split DVE / GPSIMD
        if t == CTR:
            return
        dv = Dt[t][:]
        if MD > 0:
            nc.vector.tensor_mul(dv[:, 0 : MD * WC], P2[:, 0 : MD * WC],
                                 dv[:, 0 : MD * WC])
        if MD < CF:
            nc.gpsimd.tensor_mul(dv[:, MD * WC :], P2[:, MD * WC :],
                                 dv[:, MD * WC :])

    def stage_r(t):     # tree-reduce, +asp, exp
        T = spool.tile([128, 4 * WC], FP16, tag="T", bufs=6,
                       name=f"T_{hb}_{wck}_{t}")
        Tt[t] = T
        if t == CTR:
            nc.gpsimd.memset(T[:, 3 * WC : 4 * WC], 1.0)
            return
        i, j = taps[t]
        dv = Dt.pop(t)[:]
        nc.vector.tensor_add(dv[:, 0 : 4 * WC], dv[:, 0 : 4 * WC],
                             dv[:, 4 * WC : 8 * WC])
        nc.vector.tensor_add(dv[:, 0 : 2 * WC], dv[:, 0 : 2 * WC],
                             dv[:, 2 * WC : 4 * WC])
        nc.vector.tensor_add(dv[:, 0:WC], dv[:, 0:WC], dv[:, WC : 2 * WC])
        ap = asp_ap(i, j)
        if ap is not None:
            nc.vector.tensor_add(dv[:, 0:WC], dv[:, 0:WC], ap)
        nc.scalar.activation(T[:, 3 * WC : 4 * WC], dv[:, 0:WC], AF.Exp,
                             scale=-1.0)

    def stage_c(t):     # w*fn3, then PE accumulates [w*fn3 | w] into PSUM
        i, j = taps[t]
        T = Tt.pop(t)
        fn3 = _cm(F[i][:])[:, 0:CO, j : j + WC]
        if t == CTR:
            nc.vector.tensor_copy(_cm(T[:, 0 : CO * WC], WC, CO), fn3)
        else:
            w_b = T[:, 3 * WC : 4 * WC].unsqueeze(1).broadcast_to(
                [128, CO, WC])
            eng = nc.gpsimd if (GP_MUL3 and t % GP_MUL3 == 0) else nc.vector
            eng.tensor_mul(_cm(T[:, 0 : CO * WC], WC, CO), w_b, fn3)
        for k in range(4):
            nc.tensor.matmul(
                ps[:, k * WC : (k + 1) * WC], ident[:],
                T[:, k * WC : (k + 1) * WC],
                start=(t == 0), stop=(t == n - 1))

    # issue order inside an iteration matters: exp (in stage_r) must precede
    # the next square (stage_a) in the ACT queue, else everything downstream
    # of exp serializes behind the 4.4us square.
    for t in range(n + 3):
        if 0 <= t - 2 < n:
            stage_r(t - 2)
        if t < n:
            stage_a(t)
        if 0 <= t - 1 < n:
            stage_m(t - 1)
        if 0 <= t - 3 < n:
            stage_c(t - 3)

    # ---- out = acc / wsum ----
    rec = spool.tile([128, WC], FP32, tag="rec", name=f"rec_{hb}_{wck}")
    nc.vector.reciprocal(rec[:], ps[:, 3 * WC : 4 * WC])
    out3 = spool.tile([128, CO * WC], FP32, tag="out3", name=f"o3_{hb}_{wck}")
    rec_b = rec[:].unsqueeze(1).broadcast_to([128, CO, WC])
    nc.vector.tensor_mul(_cm(out3[:], WC, CO), rec_b,
                         _cm(ps[:, 0 : CO * WC], WC, CO))
    for c in range(CO):
        nc.sync.dma_start(out=y[c, r0 : r0 + 128, w0 : w0 + WC],
                          in_=out3[:, c * WC : (c + 1) * WC])


def shard_inputs(input):
    """input [2,18,1024,1024] -> 8 per-core slabs [262, 18, 1024]."""
    input = np.asarray(input, dtype=np.float32)
    per_b = 4
    rows = H // per_b
    in_maps = []
    for core in range(8):
        b, q = divmod(core, per_b)
        r0 = q * rows
        slab = np.full((HIN, C_ALL, W), SENT, dtype=np.float32)
        s_lo = max(r0 - RAD, 0)
        s_hi = min(r0 + rows + RAD, H)
        slab[s_lo - (r0 - RAD) : s_hi - (r0 - RAD), :, :] = (
            input[b, :, s_lo:s_hi, :].transpose(1, 0, 2))
        in_maps.append({"x": np.ascontiguousarray(slab)})
    return in_maps


def assemble(results):
    out = np.empty((B, CO, H, W), dtype=np.float32)
    rows = H // 4
    for core in range(8):
        b, q = divmod(core, 4)
        out[b, :, q * rows : (q + 1) * rows, :] = results[core]["y"]
    return out


def kernel(input):
    from concourse.bass_utils import run_bass_kernel_spmd

    if "nc" not in _CACHED:
        _CACHED["nc"] = build_nc()
    in_maps = shard_inputs(input)
    res = run_bass_kernel_spmd(_CACHED["nc"], in_maps, list(range(8)))
    return assemble(res.results)
